# revision 13
# baseline (speedup 1.0000x reference)
"""Multi-Head Latent Attention (MLA) on 8 Trainium2 NeuronCores.

Sharding: core = b*4 + hg, b in {0,1} batch, hg in 0..3 head-groups of 4
heads (512 of the 2048 d_out dims). The latent projections (c_kv) are
computed per-core; the low-rank Q path is absorbed on device:
    W_effQ^T = W_DQ^T @ W_UQ_shard^T   ([d_in, 512])
so q_shard = x_b @ W_effQ (one 2048-contraction matmul instead of the
replicated full c_q).

Everything on device lives in transposed "feature-on-partition" layout:
  XT = x[b]^T [d_in, N], QT = q^T, CKT = c_kv^T, KT = k^T. Attention
computes S^T tiles [ktok, qtok] directly (matmul lhsT=KT-slice,
rhs=QT-slice), so softmax probabilities come out of exp already in the
layout the ctx matmul needs (contraction over ktok on partitions) — no
PE transposes. The softmax denominator is a ones-vector matmul
accumulated alongside ctx; normalization is applied to ctx^T via a PE
outer-product broadcast of 1/sum. Causality: affine_select zeroes
P^T[kj, q] for kj > q after exp (no max-subtraction needed: scores are
O(1) by construction).

Output per core: partial out^T [d_in, N] (contraction over this core's
512 ctx dims); host sums the 4 head-group partials per batch and adds
the bias.
"""

import math
from contextlib import ExitStack

import numpy as np

import concourse.bass as bass
import concourse.mybir as mybir
import concourse.tile as tile
from concourse.bass_utils import run_bass_kernel_spmd
from concourse.vector_clock import ScopedClock, VectorClock

FP32 = mybir.dt.float32
FP32R = mybir.dt.float32r
P = 128
CH = 512


class SplitDrainTileContext(tile.TileContext):
    """TileContext whose tail drain splits sem waits across multiple NOPs.

    The walrus build in this container rejects instructions carrying >2
    sync waits ("Too many sync wait commands"); stock TileContext puts a
    wait for every outstanding proc on the single kernel-tail drain.
    """

    def _drain_and_barrier(self, tick_clock, wait_clock):
        g = tick_clock.global_clock
        n = len(g)
        for i in range(n):
            t = g[i]
            if t <= 0:
                continue
            vc = VectorClock([0] * n)
            vc.require_at_least(i, t)
            nop = self.nc.sync.nop(hint="split_drain_wait", nofuse=True)
            wait_clock.add_sem_waits(nop.ins, ScopedClock({None: vc}))
        self.nc.sync.drain()
        self.nc.all_engine_barrier()
        assert self.sems is not None
        popped = self.nc._tile_sem_poison_stack.pop()
        assert popped is self._sem_poison
        self.nc.clear_and_free_semaphores(list(self.sems.allocated().values()))
        self.nc.all_engine_barrier()


def split_multi_waits(nc, max_waits=1):
    """Hoist extra sync waits onto same-engine NOPs.

    The walrus build here rejects instructions with more than ~2 sync wait
    commands; Tile freely attaches one wait per outstanding proc. An engine
    executes its stream in order, so a NOP carrying a wait immediately
    before the instruction is semantically identical.
    """
    for fn in nc.m.functions:
        for bb in fn.blocks:
            new_insts = []
            changed = False
            for inst in bb.instructions:
                si = inst.sync_info
                waits = list(si.on_wait) if si is not None else []
                if len(waits) > max_waits:
                    extra, keep = waits[:-max_waits], waits[-max_waits:]
                    for k, w in enumerate(extra):
                        nop = mybir.InstNoOp(
                            name=f"{inst.name}.w{k}",
                            sync_info=mybir.SyncInfo(on_wait=[w], on_update=[]),
                            bass_nofuse=True,
                            engine=inst.engine,
                        )
                        new_insts.append(nop)
                    inst.sync_info = mybir.SyncInfo(
                        on_wait=keep, on_update=list(si.on_update)
                    )
                    changed = True
                new_insts.append(inst)
            if changed:
                bb.instructions = new_insts


def build_nc(N=2048, D=2048, QL=2048, KV=512, HC=4, DH=128, split=True):
    """Build the per-core Bass program (identical on all 8 cores)."""
    HD = HC * DH  # this core's slice of d_out
    n_ct = D // P  # d_in partition tiles
    n_lt = QL // P  # q-latent tiles (W_effQ contraction)
    n_klt = KV // P  # kv-latent tiles
    n_ht = HD // P  # head tiles (DH == P so one tile per head)
    n_ch = N // CH  # token chunks
    kpc = CH // P  # ktiles per chunk (4)
    scale = 1.0 / math.sqrt(DH)
    assert DH == P and n_ct % 4 == 0

    nc = bass.Bass("TRN2", target_bir_lowering=False, debug=False)
    xt = nc.declare_dram_parameter("xt", [D, N], FP32, isOutput=False)
    wdq = nc.declare_dram_parameter("wdq", [QL, D], FP32, isOutput=False)
    wuqt = nc.declare_dram_parameter("wuqt", [QL, HD], FP32, isOutput=False)
    wdkvt = nc.declare_dram_parameter("wdkvt", [D, KV], FP32, isOutput=False)
    wukt = nc.declare_dram_parameter("wukt", [KV, HD], FP32, isOutput=False)
    wuvt = nc.declare_dram_parameter("wuvt", [KV, HD], FP32, isOutput=False)
    wot = nc.declare_dram_parameter("wot", [HD, D], FP32, isOutput=False)
    ones_d = nc.declare_dram_parameter("ones", [P, P], FP32, isOutput=False)
    outt = nc.declare_dram_parameter("outt", [D, N], FP32, isOutput=True)

    with SplitDrainTileContext(nc) as tc, ExitStack() as top:
        mm = nc.tensor.matmul
        dram = top.enter_context(tc.tile_pool(name="dram", bufs=1, space="DRAM"))
        qt_spill = dram.tile([HD, N], FP32, tag="qts", name="qt_spill")
        ot_spill = dram.tile([HD, N], FP32, tag="ots", name="ot_spill")

        const = top.enter_context(tc.tile_pool(name="const", bufs=1))
        ones_k = const.tile([P, 1], FP32R, tag="ones_k", name="ones_k")
        nc.sync.dma_start(out=ones_k, in_=ones_d[:, :1].bitcast(FP32R))
        ones_1 = const.tile([1, P], FP32R, tag="ones_1", name="ones_1")
        nc.sync.dma_start(out=ones_1, in_=ones_d[:1, :].bitcast(FP32R))

        ps = top.enter_context(tc.tile_pool(name="ps", bufs=8, space="PSUM"))

        def pst(name):
            return ps.tile([P, CH], FP32, tag="ps", name=name)

        kvp = top.enter_context(tc.tile_pool(name="kv", bufs=1))
        kt_sb = [
            kvp.tile([P, N], FP32R, tag=f"kt{h}", name=f"kt{h}") for h in range(n_ht)
        ]
        v_sb = [
            kvp.tile([P, HD], FP32R, tag=f"v{t}", name=f"v{t}")
            for t in range(N // P)
        ]

        # ---- Phase W: W_effQ^T [D, HD] = W_DQ^T @ W_UQ_shard^T -------------
        with tc.tile_pool(name="wqt", bufs=1) as wqtp:
            wqt = [
                wqtp.tile([P, HD], FP32R, tag=f"wqt{c}", name=f"wqt{c}")
                for c in range(n_ct)
            ]
            with (
                tc.tile_pool(name="wuqtp", bufs=1) as wuqtp,
                tc.tile_pool(name="wdqs", bufs=3) as wdqs,
            ):
                wuqt_sb = []
                for lt in range(n_lt):
                    w = wuqtp.tile([P, HD], FP32R, tag=f"wuqt{lt}", name=f"wuqt{lt}")
                    nc.sync.dma_start(out=w, in_=wuqt[lt * P : (lt + 1) * P, :].bitcast(FP32R))
                    wuqt_sb.append(w)
                for cb in range(n_ct // 4):
                    pss = [pst(f"psw{cb}_{i}") for i in range(4)]
                    for lt in range(n_lt):
                        wd = wdqs.tile([P, 4 * P], FP32R, tag="wdq", name=f"wdq{cb}_{lt}")
                        nc.sync.dma_start(
                            out=wd,
                            in_=wdq[lt * P : (lt + 1) * P, cb * 4 * P : (cb + 1) * 4 * P].bitcast(FP32R),
                        )
                        for ci in range(4):
                            mm(
                                pss[ci][:, :HD],
                                lhsT=wd[:, ci * P : (ci + 1) * P],
                                rhs=wuqt_sb[lt],
                                start=(lt == 0),
                                stop=(lt == n_lt - 1),
                            )
                    for ci in range(4):
                        nc.scalar.copy(out=wqt[cb * 4 + ci], in_=pss[ci][:, :HD])

            # ---- Phase X/KV (per token chunk): QT, CKT, KT, V -------------
            with (
                tc.tile_pool(name="wdkvtp", bufs=1) as wdkvtp,
                tc.tile_pool(name="wukvp", bufs=1) as wukvp,
                tc.tile_pool(name="xtp", bufs=1) as xtp,
                tc.tile_pool(name="cktp", bufs=2) as cktp,
                tc.tile_pool(name="stg", bufs=3) as stg,
            ):
                wdkvt_sb = []
                for ct in range(n_ct):
                    w = wdkvtp.tile([P, KV], FP32R, tag=f"wdkvt{ct}", name=f"wdkvt{ct}")
                    nc.sync.dma_start(out=w, in_=wdkvt[ct * P : (ct + 1) * P, :].bitcast(FP32R))
                    wdkvt_sb.append(w)
                wukt_sb, wuvt_sb = [], []
                for kl in range(n_klt):
                    w = wukvp.tile([P, HD], FP32R, tag=f"wukt{kl}", name=f"wukt{kl}")
                    nc.sync.dma_start(out=w, in_=wukt[kl * P : (kl + 1) * P, :].bitcast(FP32R))
                    wukt_sb.append(w)
                    w = wukvp.tile([P, HD], FP32R, tag=f"wuvt{kl}", name=f"wuvt{kl}")
                    nc.sync.dma_start(out=w, in_=wuvt[kl * P : (kl + 1) * P, :].bitcast(FP32R))
                    wuvt_sb.append(w)

                for ch in range(n_ch):
                    tok = slice(ch * CH, (ch + 1) * CH)
                    xts = []
                    for ct in range(n_ct):
                        x_t = xtp.tile([P, CH], FP32R, tag=f"xt{ct}", name=f"xt{ct}_{ch}")
                        nc.sync.dma_start(out=x_t, in_=xt[ct * P : (ct + 1) * P, tok].bitcast(FP32R))
                        xts.append(x_t)
                    # QT chunk
                    psq = [pst(f"psq{ch}_{q}") for q in range(n_ht)]
                    for ct in range(n_ct):
                        for q in range(n_ht):
                            mm(
                                psq[q],
                                lhsT=wqt[ct][:, q * P : (q + 1) * P],
                                rhs=xts[ct],
                                start=(ct == 0),
                                stop=(ct == n_ct - 1),
                            )
                    for q in range(n_ht):
                        st = stg.tile([P, CH], FP32, tag="stg", name=f"stq{ch}_{q}")
                        nc.scalar.copy(out=st, in_=psq[q])
                        nc.sync.dma_start(
                            out=qt_spill[q * P : (q + 1) * P, tok], in_=st
                        )
                    # CKT chunk
                    psc = [pst(f"psc{ch}_{k}") for k in range(n_klt)]
                    for ct in range(n_ct):
                        for k in range(n_klt):
                            mm(
                                psc[k],
                                lhsT=wdkvt_sb[ct][:, k * P : (k + 1) * P],
                                rhs=xts[ct],
                                start=(ct == 0),
                                stop=(ct == n_ct - 1),
                            )
                    ckt = []
                    for k in range(n_klt):
                        c_t = cktp.tile([P, CH], FP32R, tag=f"ckt{k}", name=f"ckt{k}_{ch}")
                        nc.scalar.copy(out=c_t, in_=psc[k])
                        ckt.append(c_t)
                    # KT chunk (contraction over kv-latent)
                    psk = [pst(f"psk{ch}_{h}") for h in range(n_ht)]
                    for kl in range(n_klt):
                        for h in range(n_ht):
                            mm(
                                psk[h],
                                lhsT=wukt_sb[kl][:, h * P : (h + 1) * P],
                                rhs=ckt[kl],
                                start=(kl == 0),
                                stop=(kl == n_klt - 1),
                            )
                    for h in range(n_ht):
                        nc.scalar.copy(out=kt_sb[h][:, tok], in_=psk[h])
                    # V chunk: token-major [tok, HD]
                    for tt in range(kpc):
                        tglob = ch * kpc + tt
                        psv = pst(f"psv{tglob}")
                        for kl in range(n_klt):
                            mm(
                                psv[:, :HD],
                                lhsT=ckt[kl][:, tt * P : (tt + 1) * P],
                                rhs=wuvt_sb[kl],
                                start=(kl == 0),
                                stop=(kl == n_klt - 1),
                            )
                        nc.scalar.copy(out=v_sb[tglob], in_=psv[:, :HD])

        # ---- Phase A: causal attention per (head, qgroup) ------------------
        with (
            tc.tile_pool(name="wotp", bufs=1) as wotp,
            tc.tile_pool(name="qld", bufs=2) as qld,
            tc.tile_pool(name="ptp", bufs=4) as ptp,
            tc.tile_pool(name="bcp", bufs=2) as bcp,
            tc.tile_pool(name="recp", bufs=2) as recp,
            tc.tile_pool(name="ostg", bufs=2) as ostg,
            tc.tile_pool(name="old", bufs=2) as old,
            tc.tile_pool(name="oout", bufs=3) as oout,
        ):
            wot_sb = []
            for d in range(n_ht):
                w = wotp.tile([P, D], FP32R, tag=f"wot{d}", name=f"wot{d}")
                nc.sync.dma_start(out=w, in_=wot[d * P : (d + 1) * P, :].bitcast(FP32R))
                wot_sb.append(w)

            for h in range(n_ht):
                for g in range(n_ch):
                    qg = slice(g * CH, (g + 1) * CH)
                    q_t = qld.tile([P, CH], FP32R, tag="qld", name=f"q{h}_{g}")
                    nc.sync.dma_start(
                        out=q_t, in_=qt_spill[h * P : (h + 1) * P, qg].bitcast(FP32R)
                    )
                    nk = kpc * (g + 1)
                    ps_ot = pst(f"psot{h}_{g}")
                    ps_cs = pst(f"pscs{h}_{g}")
                    for t in range(nk):
                        ps_s = pst(f"pss{h}_{g}_{t}")
                        mm(
                            ps_s,
                            lhsT=kt_sb[h][:, t * P : (t + 1) * P],
                            rhs=q_t,
                            start=True,
                            stop=True,
                        )
                        pt = ptp.tile([P, CH], FP32R, tag="pt", name=f"pt{h}_{g}_{t}")
                        nc.scalar.activation(
                            out=pt,
                            in_=ps_s,
                            func=mybir.ActivationFunctionType.Exp,
                            scale=scale,
                        )
                        j = t - kpc * g
                        if j >= 0:
                            # keep P^T[kj, q] only where global q >= global kj
                            nc.gpsimd.affine_select(
                                out=pt,
                                in_=pt,
                                compare_op=mybir.AluOpType.is_ge,
                                fill=0.0,
                                base=-P * j,
                                channel_multiplier=-1,
                                pattern=[[1, CH]],
                            )
                        mm(
                            ps_cs[:1, :],
                            lhsT=ones_k,
                            rhs=pt,
                            start=(t == 0),
                            stop=(t == nk - 1),
                        )
                        mm(
                            ps_ot,
                            lhsT=v_sb[t][:, h * P : (h + 1) * P],
                            rhs=pt,
                            start=(t == 0),
                            stop=(t == nk - 1),
                        )
                    rec = recp.tile([1, CH], FP32R, tag="rec", name=f"rec{h}_{g}")
                    with nc.allow_low_precision(reason="fp32r softmax recip"):
                        nc.vector.reciprocal(out=rec, in_=ps_cs[:1, :])
                    ps_bc = pst(f"psbc{h}_{g}")
                    mm(ps_bc, lhsT=ones_1, rhs=rec, start=True, stop=True)
                    bc = bcp.tile([P, CH], FP32, tag="bc", name=f"bc{h}_{g}")
                    nc.scalar.copy(out=bc, in_=ps_bc)
                    ot_t = ostg.tile([P, CH], FP32, tag="ostg", name=f"ot{h}_{g}")
                    nc.vector.tensor_mul(out=ot_t, in0=ps_ot, in1=bc)
                    nc.sync.dma_start(
                        out=ot_spill[h * P : (h + 1) * P, qg], in_=ot_t
                    )

            # ---- Phase O: out^T += W_O_shard^T-contraction over ctx dims ---
            for ch in range(n_ch):
                tok = slice(ch * CH, (ch + 1) * CH)
                ots = []
                for d in range(n_ht):
                    o_t = old.tile([P, CH], FP32R, tag=f"ol{d}", name=f"ol{d}_{ch}")
                    nc.sync.dma_start(
                        out=o_t, in_=ot_spill[d * P : (d + 1) * P, tok].bitcast(FP32R)
                    )
                    ots.append(o_t)
                for ct in range(n_ct):
                    ps_o = pst(f"pso{ch}_{ct}")
                    for d in range(n_ht):
                        mm(
                            ps_o,
                            lhsT=wot_sb[d][:, ct * P : (ct + 1) * P],
                            rhs=ots[d],
                            start=(d == 0),
                            stop=(d == n_ht - 1),
                        )
                    oo = oout.tile([P, CH], FP32, tag="oo", name=f"oo{ch}_{ct}")
                    nc.vector.tensor_copy(out=oo, in_=ps_o)
                    nc.sync.dma_start(out=outt[ct * P : (ct + 1) * P, tok], in_=oo)

    if split:
        # for walrus only; CoreSim's race detector can't see the added NOPs
        split_multi_waits(nc)
    return nc


# ---------------------------------------------------------------------------
# Host side
# ---------------------------------------------------------------------------
B, N, D_IN = 2, 2048, 2048
D_OUT, N_HEADS = 2048, 16
D_C_KV, D_C_Q = 512, 2048
N_CORES = 8
HG = 4  # head-groups
HD = D_OUT // HG  # 512 dims per head-group

_NC_CACHE = {}


def _get_nc():
    if "nc" not in _NC_CACHE:
        _NC_CACHE["nc"] = build_nc(
            N=N, D=D_IN, QL=D_C_Q, KV=D_C_KV, HC=N_HEADS // HG, DH=D_OUT // N_HEADS
        )
    return _NC_CACHE["nc"]


def make_in_maps(x, W_DQ, W_UQ, W_DKV, W_UK, W_UV, W_O):
    c = np.ascontiguousarray
    xtb = [c(np.asarray(x[b], np.float32).T) for b in range(B)]
    wdq = c(np.asarray(W_DQ, np.float32))
    wdkvt = c(np.asarray(W_DKV, np.float32).T)
    in_maps = []
    for core in range(N_CORES):
        b, hg = divmod(core, HG)
        hs = slice(hg * HD, (hg + 1) * HD)
        in_maps.append(
            {
                "xt": xtb[b],
                "wdq": wdq,
                "wuqt": c(np.asarray(W_UQ, np.float32)[hs, :].T),
                "wdkvt": wdkvt,
                "wukt": c(np.asarray(W_UK, np.float32)[hs, :].T),
                "wuvt": c(np.asarray(W_UV, np.float32)[hs, :].T),
                "wot": c(np.asarray(W_O, np.float32)[:, hs].T),
                "ones": np.ones((P, P), np.float32),
            }
        )
    return in_maps


def kernel(x, W_DQ, W_UQ, W_DKV, W_UK, W_UV, W_O, b_O, _run_kwargs=None):
    nc = _get_nc()
    in_maps = make_in_maps(x, W_DQ, W_UQ, W_DKV, W_UK, W_UV, W_O)
    res = run_bass_kernel_spmd(
        nc, in_maps, list(range(N_CORES)), **(_run_kwargs or {})
    )
    out = np.zeros((B, N, D_IN), np.float32)
    for core in range(N_CORES):
        b = core // HG
        out[b] += res.results[core]["outt"].T
    out += np.asarray(b_O, np.float32)[None, None, :]
    if _run_kwargs is not None:
        _NC_CACHE["last_results"] = res
    return out


# revision 18
# speedup vs baseline: 1.0230x; 1.0230x over previous
"""Multi-Head Latent Attention (MLA) on 8 Trainium2 NeuronCores.

Sharding: core = b*4 + hg, b in {0,1} batch, hg in 0..3 head-groups of 4
heads (512 of the 2048 d_out dims). The latent projections (c_kv) are
computed per-core; the low-rank Q path is absorbed on device:
    W_effQ^T = W_DQ^T @ W_UQ_shard^T   ([d_in, 512])
so q_shard = x_b @ W_effQ (one 2048-contraction matmul instead of the
replicated full c_q).

Everything on device lives in transposed "feature-on-partition" layout:
  XT = x[b]^T [d_in, N], QT = q^T, CKT = c_kv^T, KT = k^T. Attention
computes S^T tiles [ktok, qtok] directly (matmul lhsT=KT-slice,
rhs=QT-slice), so softmax probabilities come out of exp already in the
layout the ctx matmul needs (contraction over ktok on partitions) — no
PE transposes. The softmax denominator is a ones-vector matmul
accumulated alongside ctx; normalization is applied to ctx^T via a PE
outer-product broadcast of 1/sum. Causality: affine_select zeroes
P^T[kj, q] for kj > q after exp (no max-subtraction needed: scores are
O(1) by construction).

Output per core: partial out^T [d_in, N] (contraction over this core's
512 ctx dims); host sums the 4 head-group partials per batch and adds
the bias.
"""

import math
from contextlib import ExitStack

import numpy as np

import concourse.bass as bass
import concourse.mybir as mybir
import concourse.tile as tile
from concourse.bass_utils import run_bass_kernel_spmd
from concourse.vector_clock import ScopedClock, VectorClock

FP32 = mybir.dt.float32
FP32R = mybir.dt.float32r
P = 128
CH = 512


class SplitDrainTileContext(tile.TileContext):
    """TileContext whose tail drain splits sem waits across multiple NOPs.

    The walrus build in this container rejects instructions carrying >2
    sync waits ("Too many sync wait commands"); stock TileContext puts a
    wait for every outstanding proc on the single kernel-tail drain.
    """

    def _drain_and_barrier(self, tick_clock, wait_clock):
        g = tick_clock.global_clock
        n = len(g)
        for i in range(n):
            t = g[i]
            if t <= 0:
                continue
            vc = VectorClock([0] * n)
            vc.require_at_least(i, t)
            nop = self.nc.sync.nop(hint="split_drain_wait", nofuse=True)
            wait_clock.add_sem_waits(nop.ins, ScopedClock({None: vc}))
        self.nc.sync.drain()
        self.nc.all_engine_barrier()
        assert self.sems is not None
        popped = self.nc._tile_sem_poison_stack.pop()
        assert popped is self._sem_poison
        self.nc.clear_and_free_semaphores(list(self.sems.allocated().values()))
        self.nc.all_engine_barrier()


def split_multi_waits(nc, max_waits=1):
    """Hoist extra sync waits onto same-engine NOPs.

    The walrus build here rejects instructions with more than ~2 sync wait
    commands; Tile freely attaches one wait per outstanding proc. An engine
    executes its stream in order, so a NOP carrying a wait immediately
    before the instruction is semantically identical.
    """
    for fn in nc.m.functions:
        for bb in fn.blocks:
            new_insts = []
            changed = False
            for inst in bb.instructions:
                si = inst.sync_info
                waits = list(si.on_wait) if si is not None else []
                if len(waits) > max_waits:
                    extra, keep = waits[:-max_waits], waits[-max_waits:]
                    for k, w in enumerate(extra):
                        nop = mybir.InstNoOp(
                            name=f"{inst.name}.w{k}",
                            sync_info=mybir.SyncInfo(on_wait=[w], on_update=[]),
                            bass_nofuse=True,
                            engine=inst.engine,
                        )
                        new_insts.append(nop)
                    inst.sync_info = mybir.SyncInfo(
                        on_wait=keep, on_update=list(si.on_update)
                    )
                    changed = True
                new_insts.append(inst)
            if changed:
                bb.instructions = new_insts


def build_nc(N=2048, D=2048, QL=2048, KV=512, HC=4, DH=128, split=True):
    """Build the per-core Bass program (identical on all 8 cores)."""
    HD = HC * DH  # this core's slice of d_out
    n_ct = D // P  # d_in partition tiles
    n_lt = QL // P  # q-latent tiles (W_effQ contraction)
    n_klt = KV // P  # kv-latent tiles
    n_ht = HD // P  # head tiles (DH == P so one tile per head)
    n_ch = N // CH  # token chunks
    kpc = CH // P  # ktiles per chunk (4)
    scale = 1.0 / math.sqrt(DH)
    assert DH == P and n_ct % 4 == 0

    nc = bass.Bass("TRN2", target_bir_lowering=False, debug=False)
    xt = nc.declare_dram_parameter("xt", [D, N], FP32, isOutput=False)
    wdq = nc.declare_dram_parameter("wdq", [QL, D], FP32, isOutput=False)
    wuqt = nc.declare_dram_parameter("wuqt", [QL, HD], FP32, isOutput=False)
    wdkvt = nc.declare_dram_parameter("wdkvt", [D, KV], FP32, isOutput=False)
    wukt = nc.declare_dram_parameter("wukt", [KV, HD], FP32, isOutput=False)
    wuvt = nc.declare_dram_parameter("wuvt", [KV, HD], FP32, isOutput=False)
    wot = nc.declare_dram_parameter("wot", [HD, D], FP32, isOutput=False)
    ones_d = nc.declare_dram_parameter("ones", [P, P], FP32, isOutput=False)
    outt = nc.declare_dram_parameter("outt", [D, N], FP32, isOutput=True)

    with SplitDrainTileContext(nc) as tc, ExitStack() as top:
        mm = nc.tensor.matmul
        dram = top.enter_context(tc.tile_pool(name="dram", bufs=1, space="DRAM"))
        qt_spill = dram.tile([HD, N], FP32, tag="qts", name="qt_spill")
        ot_spill = dram.tile([HD, N], FP32, tag="ots", name="ot_spill")

        const = top.enter_context(tc.tile_pool(name="const", bufs=1))
        ones_k = const.tile([P, 1], FP32R, tag="ones_k", name="ones_k")
        nc.sync.dma_start(out=ones_k, in_=ones_d[:, :1].bitcast(FP32R))
        ones_1 = const.tile([1, P], FP32R, tag="ones_1", name="ones_1")
        nc.sync.dma_start(out=ones_1, in_=ones_d[:1, :].bitcast(FP32R))

        kvp = top.enter_context(tc.tile_pool(name="kv", bufs=1))
        kt_sb = [
            kvp.tile([P, N], FP32R, tag=f"kt{h}", name=f"kt{h}") for h in range(n_ht)
        ]
        v_sb = [
            kvp.tile([P, HD], FP32R, tag=f"v{t}", name=f"v{t}")
            for t in range(N // P)
        ]

        # ---- Phase W: W_effQ^T [D, HD] = W_DQ^T @ W_UQ_shard^T -------------
        with tc.tile_pool(name="wqt", bufs=1) as wqtp:
            wqt = [
                wqtp.tile([P, HD], FP32R, tag=f"wqt{c}", name=f"wqt{c}")
                for c in range(n_ct)
            ]
            with (
                tc.tile_pool(name="wuqtp", bufs=1) as wuqtp,
                tc.tile_pool(name="wdqs", bufs=3) as wdqs,
                tc.tile_pool(name="psW", bufs=4, space="PSUM") as psW,
            ):
                wuqt_sb = [None] * n_lt
                for cb in range(n_ct // 4):
                    pss = [
                        psW.tile([P, CH], FP32, tag="psW", name=f"psw{cb}_{i}")
                        for i in range(4)
                    ]
                    for lt in range(n_lt):
                        if cb == 0:
                            # first use drives the DMA order: lt-ascending
                            w = wuqtp.tile(
                                [P, HD], FP32R, tag=f"wuqt{lt}", name=f"wuqt{lt}"
                            )
                            nc.sync.dma_start(
                                out=w,
                                in_=wuqt[lt * P : (lt + 1) * P, :].bitcast(FP32R),
                            )
                            wuqt_sb[lt] = w
                        wd = wdqs.tile([P, 4 * P], FP32R, tag="wdq", name=f"wdq{cb}_{lt}")
                        nc.sync.dma_start(
                            out=wd,
                            in_=wdq[lt * P : (lt + 1) * P, cb * 4 * P : (cb + 1) * 4 * P].bitcast(FP32R),
                        )
                        for ci in range(4):
                            mm(
                                pss[ci][:, :HD],
                                lhsT=wd[:, ci * P : (ci + 1) * P],
                                rhs=wuqt_sb[lt],
                                start=(lt == 0),
                                stop=(lt == n_lt - 1),
                            )
                    for ci in range(4):
                        nc.scalar.copy(out=wqt[cb * 4 + ci], in_=pss[ci][:, :HD])

            # ---- Phase X/KV (per token chunk): QT, CKT, KT, V -------------
            with (
                tc.tile_pool(name="wdkvtp", bufs=1) as wdkvtp,
                tc.tile_pool(name="wukvp", bufs=1) as wukvp,
                tc.tile_pool(name="xtp", bufs=1) as xtp,
                tc.tile_pool(name="cktp", bufs=2) as cktp,
                tc.tile_pool(name="stg", bufs=3) as stg,
                tc.tile_pool(name="psX", bufs=8, space="PSUM") as psX,
            ):

                def pst(name):
                    return psX.tile([P, CH], FP32, tag="psX", name=name)

                wdkvt_sb = []
                for ct in range(n_ct):
                    w = wdkvtp.tile([P, KV], FP32R, tag=f"wdkvt{ct}", name=f"wdkvt{ct}")
                    nc.sync.dma_start(out=w, in_=wdkvt[ct * P : (ct + 1) * P, :].bitcast(FP32R))
                    wdkvt_sb.append(w)
                wukt_sb, wuvt_sb = [], []
                for kl in range(n_klt):
                    w = wukvp.tile([P, HD], FP32R, tag=f"wukt{kl}", name=f"wukt{kl}")
                    nc.sync.dma_start(out=w, in_=wukt[kl * P : (kl + 1) * P, :].bitcast(FP32R))
                    wukt_sb.append(w)
                    w = wukvp.tile([P, HD], FP32R, tag=f"wuvt{kl}", name=f"wuvt{kl}")
                    nc.sync.dma_start(out=w, in_=wuvt[kl * P : (kl + 1) * P, :].bitcast(FP32R))
                    wuvt_sb.append(w)

                for ch in range(n_ch):
                    tok = slice(ch * CH, (ch + 1) * CH)
                    xts = []
                    for ct in range(n_ct):
                        x_t = xtp.tile([P, CH], FP32R, tag=f"xt{ct}", name=f"xt{ct}_{ch}")
                        nc.sync.dma_start(out=x_t, in_=xt[ct * P : (ct + 1) * P, tok].bitcast(FP32R))
                        xts.append(x_t)
                    # QT chunk
                    psq = [pst(f"psq{ch}_{q}") for q in range(n_ht)]
                    for ct in range(n_ct):
                        for q in range(n_ht):
                            mm(
                                psq[q],
                                lhsT=wqt[ct][:, q * P : (q + 1) * P],
                                rhs=xts[ct],
                                start=(ct == 0),
                                stop=(ct == n_ct - 1),
                            )
                    for q in range(n_ht):
                        st = stg.tile([P, CH], FP32, tag="stg", name=f"stq{ch}_{q}")
                        nc.vector.tensor_copy(out=st, in_=psq[q])
                        nc.sync.dma_start(
                            out=qt_spill[q * P : (q + 1) * P, tok], in_=st
                        )
                    # CKT chunk
                    psc = [pst(f"psc{ch}_{k}") for k in range(n_klt)]
                    for ct in range(n_ct):
                        for k in range(n_klt):
                            mm(
                                psc[k],
                                lhsT=wdkvt_sb[ct][:, k * P : (k + 1) * P],
                                rhs=xts[ct],
                                start=(ct == 0),
                                stop=(ct == n_ct - 1),
                            )
                    ckt = []
                    for k in range(n_klt):
                        c_t = cktp.tile([P, CH], FP32R, tag=f"ckt{k}", name=f"ckt{k}_{ch}")
                        nc.vector.tensor_copy(out=c_t, in_=psc[k])
                        ckt.append(c_t)
                    # KT chunk (contraction over kv-latent)
                    psk = [pst(f"psk{ch}_{h}") for h in range(n_ht)]
                    for kl in range(n_klt):
                        for h in range(n_ht):
                            mm(
                                psk[h],
                                lhsT=wukt_sb[kl][:, h * P : (h + 1) * P],
                                rhs=ckt[kl],
                                start=(kl == 0),
                                stop=(kl == n_klt - 1),
                            )
                    for h in range(n_ht):
                        nc.vector.tensor_copy(out=kt_sb[h][:, tok], in_=psk[h])
                    # V chunk: token-major [tok, HD]
                    for tt in range(kpc):
                        tglob = ch * kpc + tt
                        psv = pst(f"psv{tglob}")
                        for kl in range(n_klt):
                            mm(
                                psv[:, :HD],
                                lhsT=ckt[kl][:, tt * P : (tt + 1) * P],
                                rhs=wuvt_sb[kl],
                                start=(kl == 0),
                                stop=(kl == n_klt - 1),
                            )
                        nc.vector.tensor_copy(out=v_sb[tglob], in_=psv[:, :HD])

        # ---- Phase A + O: causal attention (qgroup-outer) with the output
        # projection for chunk g emitted as soon as all heads of g are done.
        with (
            tc.tile_pool(name="wotp", bufs=1) as wotp,
            tc.tile_pool(name="qld", bufs=3) as qld,
            tc.tile_pool(name="ptp", bufs=4) as ptp,
            tc.tile_pool(name="bcp", bufs=2) as bcp,
            tc.tile_pool(name="csp", bufs=2) as csp,
            tc.tile_pool(name="ostg", bufs=2) as ostg,
            tc.tile_pool(name="old", bufs=2) as old,
            tc.tile_pool(name="oout", bufs=3) as oout,
            tc.tile_pool(name="psS", bufs=3, space="PSUM") as psS,
            tc.tile_pool(name="psA", bufs=3, space="PSUM") as psA,
            tc.tile_pool(name="psO", bufs=2, space="PSUM") as psO,
        ):
            wot_sb = []
            for d in range(n_ht):
                w = wotp.tile([P, D], FP32R, tag=f"wot{d}", name=f"wot{d}")
                nc.sync.dma_start(out=w, in_=wot[d * P : (d + 1) * P, :].bitcast(FP32R))
                wot_sb.append(w)

            def out_proj_chunk(ch):
                tok = slice(ch * CH, (ch + 1) * CH)
                ots = []
                for d in range(n_ht):
                    o_t = old.tile([P, CH], FP32R, tag=f"ol{d}", name=f"ol{d}_{ch}")
                    nc.sync.dma_start(
                        out=o_t, in_=ot_spill[d * P : (d + 1) * P, tok].bitcast(FP32R)
                    )
                    ots.append(o_t)
                for ct in range(n_ct):
                    ps_o = psO.tile([P, CH], FP32, tag="psO", name=f"pso{ch}_{ct}")
                    for d in range(n_ht):
                        mm(
                            ps_o,
                            lhsT=wot_sb[d][:, ct * P : (ct + 1) * P],
                            rhs=ots[d],
                            start=(d == 0),
                            stop=(d == n_ht - 1),
                        )
                    oo = oout.tile([P, CH], FP32, tag="oo", name=f"oo{ch}_{ct}")
                    nc.vector.tensor_copy(out=oo, in_=ps_o)
                    nc.sync.dma_start(out=outt[ct * P : (ct + 1) * P, tok], in_=oo)

            for g in range(n_ch):
                qg = slice(g * CH, (g + 1) * CH)
                nk = kpc * (g + 1)
                for h in range(n_ht):
                    q_t = qld.tile([P, CH], FP32R, tag="qld", name=f"q{h}_{g}")
                    nc.sync.dma_start(
                        out=q_t, in_=qt_spill[h * P : (h + 1) * P, qg].bitcast(FP32R)
                    )
                    ps_ot = psA.tile([P, CH], FP32, tag="psA", name=f"psot{h}_{g}")
                    ps_cs = psA.tile([P, CH], FP32, tag="psA", name=f"pscs{h}_{g}")
                    for t in range(nk):
                        ps_s = psS.tile([P, CH], FP32, tag="psS", name=f"pss{h}_{g}_{t}")
                        mm(
                            ps_s,
                            lhsT=kt_sb[h][:, t * P : (t + 1) * P],
                            rhs=q_t,
                            start=True,
                            stop=True,
                        )
                        pt = ptp.tile([P, CH], FP32R, tag="pt", name=f"pt{h}_{g}_{t}")
                        nc.scalar.activation(
                            out=pt,
                            in_=ps_s,
                            func=mybir.ActivationFunctionType.Exp,
                            scale=scale,
                        )
                        j = t - kpc * g
                        if j >= 0:
                            # keep P^T[kj, q] only where global q >= global kj
                            nc.gpsimd.affine_select(
                                out=pt,
                                in_=pt,
                                compare_op=mybir.AluOpType.is_ge,
                                fill=0.0,
                                base=-P * j,
                                channel_multiplier=-1,
                                pattern=[[1, CH]],
                            )
                        mm(
                            ps_cs[:1, :],
                            lhsT=ones_k,
                            rhs=pt,
                            start=(t == 0),
                            stop=(t == nk - 1),
                        )
                        mm(
                            ps_ot,
                            lhsT=v_sb[t][:, h * P : (h + 1) * P],
                            rhs=pt,
                            start=(t == 0),
                            stop=(t == nk - 1),
                        )
                    # normalization: broadcast the sums with a PE outer
                    # product, then a 128-lane reciprocal (a [1,512] DVE
                    # reciprocal costs 3.3us; this costs ~0.6us).
                    cs_sb = csp.tile([1, CH], FP32R, tag="cs", name=f"cs{h}_{g}")
                    nc.scalar.copy(out=cs_sb, in_=ps_cs[:1, :])
                    ps_bc = psS.tile([P, CH], FP32, tag="psS", name=f"psbc{h}_{g}")
                    mm(ps_bc, lhsT=ones_1, rhs=cs_sb, start=True, stop=True)
                    bc = bcp.tile([P, CH], FP32, tag="bc", name=f"bc{h}_{g}")
                    nc.vector.reciprocal(out=bc, in_=ps_bc)
                    ot_t = ostg.tile([P, CH], FP32, tag="ostg", name=f"ot{h}_{g}")
                    nc.vector.tensor_mul(out=ot_t, in0=ps_ot, in1=bc)
                    nc.sync.dma_start(
                        out=ot_spill[h * P : (h + 1) * P, qg], in_=ot_t
                    )
                out_proj_chunk(g)

    if split:
        # for walrus only; CoreSim's race detector can't see the added NOPs
        split_multi_waits(nc)
    return nc


# ---------------------------------------------------------------------------
# Host side
# ---------------------------------------------------------------------------
B, N, D_IN = 2, 2048, 2048
D_OUT, N_HEADS = 2048, 16
D_C_KV, D_C_Q = 512, 2048
N_CORES = 8
HG = 4  # head-groups
HD = D_OUT // HG  # 512 dims per head-group

_NC_CACHE = {}


def _get_nc():
    if "nc" not in _NC_CACHE:
        _NC_CACHE["nc"] = build_nc(
            N=N, D=D_IN, QL=D_C_Q, KV=D_C_KV, HC=N_HEADS // HG, DH=D_OUT // N_HEADS
        )
    return _NC_CACHE["nc"]


def make_in_maps(x, W_DQ, W_UQ, W_DKV, W_UK, W_UV, W_O):
    c = np.ascontiguousarray
    xtb = [c(np.asarray(x[b], np.float32).T) for b in range(B)]
    wdq = c(np.asarray(W_DQ, np.float32))
    wdkvt = c(np.asarray(W_DKV, np.float32).T)
    in_maps = []
    for core in range(N_CORES):
        b, hg = divmod(core, HG)
        hs = slice(hg * HD, (hg + 1) * HD)
        in_maps.append(
            {
                "xt": xtb[b],
                "wdq": wdq,
                "wuqt": c(np.asarray(W_UQ, np.float32)[hs, :].T),
                "wdkvt": wdkvt,
                "wukt": c(np.asarray(W_UK, np.float32)[hs, :].T),
                "wuvt": c(np.asarray(W_UV, np.float32)[hs, :].T),
                "wot": c(np.asarray(W_O, np.float32)[:, hs].T),
                "ones": np.ones((P, P), np.float32),
            }
        )
    return in_maps


def kernel(x, W_DQ, W_UQ, W_DKV, W_UK, W_UV, W_O, b_O, _run_kwargs=None):
    nc = _get_nc()
    in_maps = make_in_maps(x, W_DQ, W_UQ, W_DKV, W_UK, W_UV, W_O)
    res = run_bass_kernel_spmd(
        nc, in_maps, list(range(N_CORES)), **(_run_kwargs or {})
    )
    out = np.zeros((B, N, D_IN), np.float32)
    for core in range(N_CORES):
        b = core // HG
        out[b] += res.results[core]["outt"].T
    out += np.asarray(b_O, np.float32)[None, None, :]
    if _run_kwargs is not None:
        _NC_CACHE["last_results"] = res
    return out


# revision 20
# speedup vs baseline: 1.1186x; 1.0934x over previous
"""Multi-Head Latent Attention (MLA) on 8 Trainium2 NeuronCores.

Sharding: core = b*4 + hg, b in {0,1} batch, hg in 0..3 head-groups of 4
heads (512 of the 2048 d_out dims). The latent projections (c_kv) are
computed per-core; the low-rank Q path is absorbed on device:
    W_effQ^T = W_DQ^T @ W_UQ_shard^T   ([d_in, 512])
so q_shard = x_b @ W_effQ (one 2048-contraction matmul instead of the
replicated full c_q).

Everything on device lives in transposed "feature-on-partition" layout:
  XT = x[b]^T [d_in, N], QT = q^T, CKT = c_kv^T, KT = k^T. Attention
computes S^T tiles [ktok, qtok] directly (matmul lhsT=KT-slice,
rhs=QT-slice), so softmax probabilities come out of exp already in the
layout the ctx matmul needs (contraction over ktok on partitions) — no
PE transposes. The softmax denominator is a ones-vector matmul
accumulated alongside ctx; normalization is applied to ctx^T via a PE
outer-product broadcast of 1/sum. Causality: affine_select zeroes
P^T[kj, q] for kj > q after exp (no max-subtraction needed: scores are
O(1) by construction).

Output per core: partial out^T [d_in, N] (contraction over this core's
512 ctx dims); host sums the 4 head-group partials per batch and adds
the bias.
"""

import math
from contextlib import ExitStack

import numpy as np

import concourse.bass as bass
import concourse.mybir as mybir
import concourse.tile as tile
from concourse.bass_utils import run_bass_kernel_spmd
from concourse.vector_clock import ScopedClock, VectorClock

FP32 = mybir.dt.float32
FP32R = mybir.dt.float32r
BF16 = mybir.dt.bfloat16
P = 128
CH = 512


class SplitDrainTileContext(tile.TileContext):
    """TileContext whose tail drain splits sem waits across multiple NOPs.

    The walrus build in this container rejects instructions carrying >2
    sync waits ("Too many sync wait commands"); stock TileContext puts a
    wait for every outstanding proc on the single kernel-tail drain.
    """

    def _drain_and_barrier(self, tick_clock, wait_clock):
        g = tick_clock.global_clock
        n = len(g)
        for i in range(n):
            t = g[i]
            if t <= 0:
                continue
            vc = VectorClock([0] * n)
            vc.require_at_least(i, t)
            nop = self.nc.sync.nop(hint="split_drain_wait", nofuse=True)
            wait_clock.add_sem_waits(nop.ins, ScopedClock({None: vc}))
        self.nc.sync.drain()
        self.nc.all_engine_barrier()
        assert self.sems is not None
        popped = self.nc._tile_sem_poison_stack.pop()
        assert popped is self._sem_poison
        self.nc.clear_and_free_semaphores(list(self.sems.allocated().values()))
        self.nc.all_engine_barrier()


def split_multi_waits(nc, max_waits=1):
    """Hoist extra sync waits onto same-engine NOPs.

    The walrus build here rejects instructions with more than ~2 sync wait
    commands; Tile freely attaches one wait per outstanding proc. An engine
    executes its stream in order, so a NOP carrying a wait immediately
    before the instruction is semantically identical.
    """
    for fn in nc.m.functions:
        for bb in fn.blocks:
            new_insts = []
            changed = False
            for inst in bb.instructions:
                si = inst.sync_info
                waits = list(si.on_wait) if si is not None else []
                if len(waits) > max_waits:
                    extra, keep = waits[:-max_waits], waits[-max_waits:]
                    for k, w in enumerate(extra):
                        nop = mybir.InstNoOp(
                            name=f"{inst.name}.w{k}",
                            sync_info=mybir.SyncInfo(on_wait=[w], on_update=[]),
                            bass_nofuse=True,
                            engine=inst.engine,
                        )
                        new_insts.append(nop)
                    inst.sync_info = mybir.SyncInfo(
                        on_wait=keep, on_update=list(si.on_update)
                    )
                    changed = True
                new_insts.append(inst)
            if changed:
                bb.instructions = new_insts


def build_nc(N=2048, D=2048, QL=2048, KV=512, HC=4, DH=128, split=True):
    """Build the per-core Bass program (identical on all 8 cores)."""
    HD = HC * DH  # this core's slice of d_out
    n_ct = D // P  # d_in partition tiles
    n_lt = QL // P  # q-latent tiles (W_effQ contraction)
    n_klt = KV // P  # kv-latent tiles
    n_ht = HD // P  # head tiles (DH == P so one tile per head)
    n_ch = N // CH  # token chunks
    kpc = CH // P  # ktiles per chunk (4)
    scale = 1.0 / math.sqrt(DH)
    assert DH == P and n_ct % 4 == 0

    nc = bass.Bass("TRN2", target_bir_lowering=False, debug=False)
    xt = nc.declare_dram_parameter("xt", [D, N], BF16, isOutput=False)
    wdq = nc.declare_dram_parameter("wdq", [QL, D], BF16, isOutput=False)
    wuqt = nc.declare_dram_parameter("wuqt", [QL, HD], BF16, isOutput=False)
    wdkvt = nc.declare_dram_parameter("wdkvt", [D, KV], BF16, isOutput=False)
    wukt = nc.declare_dram_parameter("wukt", [KV, HD], BF16, isOutput=False)
    wuvt = nc.declare_dram_parameter("wuvt", [KV, HD], BF16, isOutput=False)
    wot = nc.declare_dram_parameter("wot", [HD, D], BF16, isOutput=False)
    ones_d = nc.declare_dram_parameter("ones", [P, P], BF16, isOutput=False)
    outt = nc.declare_dram_parameter("outt", [D, N], FP32, isOutput=True)

    with SplitDrainTileContext(nc) as tc, ExitStack() as top:
        mm = nc.tensor.matmul
        dram = top.enter_context(tc.tile_pool(name="dram", bufs=1, space="DRAM"))
        qt_spill = dram.tile([HD, N], BF16, tag="qts", name="qt_spill")
        ot_spill = dram.tile([HD, N], BF16, tag="ots", name="ot_spill")

        const = top.enter_context(tc.tile_pool(name="const", bufs=1))
        ones_k = const.tile([P, 1], BF16, tag="ones_k", name="ones_k")
        nc.sync.dma_start(out=ones_k, in_=ones_d[:, :1])
        ones_1 = const.tile([1, P], BF16, tag="ones_1", name="ones_1")
        nc.sync.dma_start(out=ones_1, in_=ones_d[:1, :])

        kvp = top.enter_context(tc.tile_pool(name="kv", bufs=1))
        kt_sb = [
            kvp.tile([P, N], BF16, tag=f"kt{h}", name=f"kt{h}") for h in range(n_ht)
        ]
        v_sb = [
            kvp.tile([P, HD], BF16, tag=f"v{t}", name=f"v{t}")
            for t in range(N // P)
        ]

        # ---- Phase W: W_effQ^T [D, HD] = W_DQ^T @ W_UQ_shard^T -------------
        with tc.tile_pool(name="wqt", bufs=1) as wqtp:
            wqt = [
                wqtp.tile([P, HD], BF16, tag=f"wqt{c}", name=f"wqt{c}")
                for c in range(n_ct)
            ]
            with (
                tc.tile_pool(name="wuqtp", bufs=1) as wuqtp,
                tc.tile_pool(name="wdqs", bufs=3) as wdqs,
                tc.tile_pool(name="psW", bufs=4, space="PSUM") as psW,
            ):
                wuqt_sb = [None] * n_lt
                for cb in range(n_ct // 4):
                    pss = [
                        psW.tile([P, CH], FP32, tag="psW", name=f"psw{cb}_{i}")
                        for i in range(4)
                    ]
                    for lt in range(n_lt):
                        if cb == 0:
                            # first use drives the DMA order: lt-ascending
                            w = wuqtp.tile(
                                [P, HD], BF16, tag=f"wuqt{lt}", name=f"wuqt{lt}"
                            )
                            nc.sync.dma_start(
                                out=w,
                                in_=wuqt[lt * P : (lt + 1) * P, :],
                            )
                            wuqt_sb[lt] = w
                        wd = wdqs.tile([P, 4 * P], BF16, tag="wdq", name=f"wdq{cb}_{lt}")
                        nc.sync.dma_start(
                            out=wd,
                            in_=wdq[lt * P : (lt + 1) * P, cb * 4 * P : (cb + 1) * 4 * P],
                        )
                        for ci in range(4):
                            mm(
                                pss[ci][:, :HD],
                                lhsT=wd[:, ci * P : (ci + 1) * P],
                                rhs=wuqt_sb[lt],
                                start=(lt == 0),
                                stop=(lt == n_lt - 1),
                            )
                    for ci in range(4):
                        nc.scalar.copy(out=wqt[cb * 4 + ci], in_=pss[ci][:, :HD])

            # ---- Phase X/KV (per token chunk): QT, CKT, KT, V -------------
            with (
                tc.tile_pool(name="wdkvtp", bufs=1) as wdkvtp,
                tc.tile_pool(name="wukvp", bufs=1) as wukvp,
                tc.tile_pool(name="xtp", bufs=1) as xtp,
                tc.tile_pool(name="cktp", bufs=2) as cktp,
                tc.tile_pool(name="stg", bufs=3) as stg,
                tc.tile_pool(name="psX", bufs=8, space="PSUM") as psX,
            ):

                def pst(name):
                    return psX.tile([P, CH], FP32, tag="psX", name=name)

                wdkvt_sb = []
                for ct in range(n_ct):
                    w = wdkvtp.tile([P, KV], BF16, tag=f"wdkvt{ct}", name=f"wdkvt{ct}")
                    nc.sync.dma_start(out=w, in_=wdkvt[ct * P : (ct + 1) * P, :])
                    wdkvt_sb.append(w)
                wukt_sb, wuvt_sb = [], []
                for kl in range(n_klt):
                    w = wukvp.tile([P, HD], BF16, tag=f"wukt{kl}", name=f"wukt{kl}")
                    nc.sync.dma_start(out=w, in_=wukt[kl * P : (kl + 1) * P, :])
                    wukt_sb.append(w)
                    w = wukvp.tile([P, HD], BF16, tag=f"wuvt{kl}", name=f"wuvt{kl}")
                    nc.sync.dma_start(out=w, in_=wuvt[kl * P : (kl + 1) * P, :])
                    wuvt_sb.append(w)

                for ch in range(n_ch):
                    tok = slice(ch * CH, (ch + 1) * CH)
                    xts = []
                    for ct in range(n_ct):
                        x_t = xtp.tile([P, CH], BF16, tag=f"xt{ct}", name=f"xt{ct}_{ch}")
                        nc.sync.dma_start(out=x_t, in_=xt[ct * P : (ct + 1) * P, tok])
                        xts.append(x_t)
                    # QT chunk
                    psq = [pst(f"psq{ch}_{q}") for q in range(n_ht)]
                    for ct in range(n_ct):
                        for q in range(n_ht):
                            mm(
                                psq[q],
                                lhsT=wqt[ct][:, q * P : (q + 1) * P],
                                rhs=xts[ct],
                                start=(ct == 0),
                                stop=(ct == n_ct - 1),
                            )
                    for q in range(n_ht):
                        st = stg.tile([P, CH], BF16, tag="stg", name=f"stq{ch}_{q}")
                        nc.vector.tensor_copy(out=st, in_=psq[q])
                        nc.sync.dma_start(
                            out=qt_spill[q * P : (q + 1) * P, tok], in_=st
                        )
                    # CKT chunk
                    psc = [pst(f"psc{ch}_{k}") for k in range(n_klt)]
                    for ct in range(n_ct):
                        for k in range(n_klt):
                            mm(
                                psc[k],
                                lhsT=wdkvt_sb[ct][:, k * P : (k + 1) * P],
                                rhs=xts[ct],
                                start=(ct == 0),
                                stop=(ct == n_ct - 1),
                            )
                    ckt = []
                    for k in range(n_klt):
                        c_t = cktp.tile([P, CH], BF16, tag=f"ckt{k}", name=f"ckt{k}_{ch}")
                        nc.vector.tensor_copy(out=c_t, in_=psc[k])
                        ckt.append(c_t)
                    # KT chunk (contraction over kv-latent)
                    psk = [pst(f"psk{ch}_{h}") for h in range(n_ht)]
                    for kl in range(n_klt):
                        for h in range(n_ht):
                            mm(
                                psk[h],
                                lhsT=wukt_sb[kl][:, h * P : (h + 1) * P],
                                rhs=ckt[kl],
                                start=(kl == 0),
                                stop=(kl == n_klt - 1),
                            )
                    for h in range(n_ht):
                        nc.vector.tensor_copy(out=kt_sb[h][:, tok], in_=psk[h])
                    # V chunk: token-major [tok, HD]
                    for tt in range(kpc):
                        tglob = ch * kpc + tt
                        psv = pst(f"psv{tglob}")
                        for kl in range(n_klt):
                            mm(
                                psv[:, :HD],
                                lhsT=ckt[kl][:, tt * P : (tt + 1) * P],
                                rhs=wuvt_sb[kl],
                                start=(kl == 0),
                                stop=(kl == n_klt - 1),
                            )
                        nc.vector.tensor_copy(out=v_sb[tglob], in_=psv[:, :HD])

        # ---- Phase A + O: causal attention (qgroup-outer) with the output
        # projection for chunk g emitted as soon as all heads of g are done.
        with (
            tc.tile_pool(name="wotp", bufs=1) as wotp,
            tc.tile_pool(name="qld", bufs=3) as qld,
            tc.tile_pool(name="ptp", bufs=4) as ptp,
            tc.tile_pool(name="bcp", bufs=2) as bcp,
            tc.tile_pool(name="csp", bufs=2) as csp,
            tc.tile_pool(name="ostg", bufs=2) as ostg,
            tc.tile_pool(name="old", bufs=2) as old,
            tc.tile_pool(name="oout", bufs=3) as oout,
            tc.tile_pool(name="psS", bufs=3, space="PSUM") as psS,
            tc.tile_pool(name="psA", bufs=3, space="PSUM") as psA,
            tc.tile_pool(name="psO", bufs=2, space="PSUM") as psO,
        ):
            wot_sb = []
            for d in range(n_ht):
                w = wotp.tile([P, D], BF16, tag=f"wot{d}", name=f"wot{d}")
                nc.sync.dma_start(out=w, in_=wot[d * P : (d + 1) * P, :])
                wot_sb.append(w)

            def out_proj_chunk(ch):
                tok = slice(ch * CH, (ch + 1) * CH)
                ots = []
                for d in range(n_ht):
                    o_t = old.tile([P, CH], BF16, tag=f"ol{d}", name=f"ol{d}_{ch}")
                    nc.sync.dma_start(
                        out=o_t, in_=ot_spill[d * P : (d + 1) * P, tok]
                    )
                    ots.append(o_t)
                for ct in range(n_ct):
                    ps_o = psO.tile([P, CH], FP32, tag="psO", name=f"pso{ch}_{ct}")
                    for d in range(n_ht):
                        mm(
                            ps_o,
                            lhsT=wot_sb[d][:, ct * P : (ct + 1) * P],
                            rhs=ots[d],
                            start=(d == 0),
                            stop=(d == n_ht - 1),
                        )
                    oo = oout.tile([P, CH], FP32, tag="oo", name=f"oo{ch}_{ct}")
                    nc.vector.tensor_copy(out=oo, in_=ps_o)
                    nc.sync.dma_start(out=outt[ct * P : (ct + 1) * P, tok], in_=oo)

            for g in range(n_ch):
                qg = slice(g * CH, (g + 1) * CH)
                nk = kpc * (g + 1)
                for h in range(n_ht):
                    q_t = qld.tile([P, CH], BF16, tag="qld", name=f"q{h}_{g}")
                    nc.sync.dma_start(
                        out=q_t, in_=qt_spill[h * P : (h + 1) * P, qg]
                    )
                    ps_ot = psA.tile([P, CH], FP32, tag="psA", name=f"psot{h}_{g}")
                    ps_cs = psA.tile([P, CH], FP32, tag="psA", name=f"pscs{h}_{g}")
                    for t in range(nk):
                        ps_s = psS.tile([P, CH], FP32, tag="psS", name=f"pss{h}_{g}_{t}")
                        mm(
                            ps_s,
                            lhsT=kt_sb[h][:, t * P : (t + 1) * P],
                            rhs=q_t,
                            start=True,
                            stop=True,
                        )
                        pt = ptp.tile([P, CH], BF16, tag="pt", name=f"pt{h}_{g}_{t}")
                        nc.scalar.activation(
                            out=pt,
                            in_=ps_s,
                            func=mybir.ActivationFunctionType.Exp,
                            scale=scale,
                        )
                        j = t - kpc * g
                        if j >= 0:
                            # keep P^T[kj, q] only where global q >= global kj
                            nc.gpsimd.affine_select(
                                out=pt,
                                in_=pt,
                                compare_op=mybir.AluOpType.is_ge,
                                fill=0.0,
                                base=-P * j,
                                channel_multiplier=-1,
                                pattern=[[1, CH]],
                            )
                        mm(
                            ps_cs[:1, :],
                            lhsT=ones_k,
                            rhs=pt,
                            start=(t == 0),
                            stop=(t == nk - 1),
                        )
                        mm(
                            ps_ot,
                            lhsT=v_sb[t][:, h * P : (h + 1) * P],
                            rhs=pt,
                            start=(t == 0),
                            stop=(t == nk - 1),
                        )
                    # normalization: broadcast the sums with a PE outer
                    # product, then a 128-lane reciprocal (a [1,512] DVE
                    # reciprocal costs 3.3us; this costs ~0.6us).
                    cs_sb = csp.tile([1, CH], BF16, tag="cs", name=f"cs{h}_{g}")
                    nc.scalar.copy(out=cs_sb, in_=ps_cs[:1, :])
                    ps_bc = psS.tile([P, CH], FP32, tag="psS", name=f"psbc{h}_{g}")
                    mm(ps_bc, lhsT=ones_1, rhs=cs_sb, start=True, stop=True)
                    bc = bcp.tile([P, CH], FP32, tag="bc", name=f"bc{h}_{g}")
                    nc.vector.reciprocal(out=bc, in_=ps_bc)
                    ot_t = ostg.tile([P, CH], BF16, tag="ostg", name=f"ot{h}_{g}")
                    nc.vector.tensor_mul(out=ot_t, in0=ps_ot, in1=bc)
                    nc.sync.dma_start(
                        out=ot_spill[h * P : (h + 1) * P, qg], in_=ot_t
                    )
                out_proj_chunk(g)

    if split:
        # for walrus only; CoreSim's race detector can't see the added NOPs
        split_multi_waits(nc)
    return nc


# ---------------------------------------------------------------------------
# Host side
# ---------------------------------------------------------------------------
B, N, D_IN = 2, 2048, 2048
D_OUT, N_HEADS = 2048, 16
D_C_KV, D_C_Q = 512, 2048
N_CORES = 8
HG = 4  # head-groups
HD = D_OUT // HG  # 512 dims per head-group

_NC_CACHE = {}


def _get_nc():
    if "nc" not in _NC_CACHE:
        _NC_CACHE["nc"] = build_nc(
            N=N, D=D_IN, QL=D_C_Q, KV=D_C_KV, HC=N_HEADS // HG, DH=D_OUT // N_HEADS
        )
    return _NC_CACHE["nc"]


def make_in_maps(x, W_DQ, W_UQ, W_DKV, W_UK, W_UV, W_O):
    import ml_dtypes

    bf = ml_dtypes.bfloat16
    c = np.ascontiguousarray

    def cb(a):
        return c(np.asarray(a, np.float32)).astype(bf)

    xtb = [cb(np.asarray(x[b], np.float32).T) for b in range(B)]
    wdq = cb(W_DQ)
    wdkvt = cb(np.asarray(W_DKV, np.float32).T)
    ones = np.ones((P, P), bf)
    in_maps = []
    for core in range(N_CORES):
        b, hg = divmod(core, HG)
        hs = slice(hg * HD, (hg + 1) * HD)
        in_maps.append(
            {
                "xt": xtb[b],
                "wdq": wdq,
                "wuqt": cb(np.asarray(W_UQ, np.float32)[hs, :].T),
                "wdkvt": wdkvt,
                "wukt": cb(np.asarray(W_UK, np.float32)[hs, :].T),
                "wuvt": cb(np.asarray(W_UV, np.float32)[hs, :].T),
                "wot": cb(np.asarray(W_O, np.float32)[:, hs].T),
                "ones": ones,
            }
        )
    return in_maps


def kernel(x, W_DQ, W_UQ, W_DKV, W_UK, W_UV, W_O, b_O, _run_kwargs=None):
    nc = _get_nc()
    in_maps = make_in_maps(x, W_DQ, W_UQ, W_DKV, W_UK, W_UV, W_O)
    res = run_bass_kernel_spmd(
        nc, in_maps, list(range(N_CORES)), **(_run_kwargs or {})
    )
    out = np.zeros((B, N, D_IN), np.float32)
    for core in range(N_CORES):
        b = core // HG
        out[b] += res.results[core]["outt"].T
    out += np.asarray(b_O, np.float32)[None, None, :]
    if _run_kwargs is not None:
        _NC_CACHE["last_results"] = res
    return out
